# revision 18
# baseline (speedup 1.0000x reference)
"""MetaRoPE kernel for Trainium2, 8 NeuronCores — hybrid DVE + TensorE.

Reference computation:
    r = rotate_m[token_positions]            # [S, D, D], block-diag 2x2 rotations
    out = einsum('bhsi,soi->bhso', x, r)     # x: [4, 32, 4096, 64] fp32

Sharding: 128 (b,h) slabs split across 8 cores (16 slabs/core). Within a
core the 16 slabs split across two independent engine pipelines:

DVE path (slabs 0..7): out = x*A + pairswap(x*B') with fp16 tables
  (see _tables); tables carry the int8 gain 127/OUT_SCALE; the final
  tensor_copy casts fp16 -> int8 (int8 TT-add writes drop DVE to 1x mode,
  a separate 2x/4x copy is cheaper).

PE path (slabs 8..15): rotation via two rounds of block-diagonal matmuls
  using the angle split s = 64q + r:  R_s = R_{64q} @ R_r.
  - pass A: 32 stationary tiles WA[m] = blockdiag(R_{2m}^T, R_{2m+1}^T);
    column (k, q) of tile m holds x[slab k, s=64q+2m+j, :] in partition
    half j. PSUM out -> ScalarE copy (fp32->fp16) into y2.
  - pass B: 64 tiles WB[q] = blockdiag(R_{64q}^T, R_{64q}^T) (both halves
    same q; columns pair the two r-parities so no partition crossing is
    needed anywhere). WB carries the 127/OUT_SCALE gain; ScalarE copies
    PSUM fp32 -> int8 codes.
  Weights are tiny (32+64 dense [128,128] fp16 tiles, ~3MB/core) vs the
  8.4MB a direct per-position-matmul would need.

I/O: x fp16 in (loads ~314 GB/s/core), out int8 (stores measured at only
~142 GB/s/core regardless of layout/rings — int8 halves the 59us fp16
store floor). Quantization is absolute (int8 code = out*127/8), and the
gate is max-abs-err / max|expected|, so int8 adds only ~4e-3.

Engine duty: Vector = DVE 3-pass + int8 cast for its slabs; Tensor =
matmuls + all store DMA triggers (HWDGE on the PE ring); Scalar = table/
weight load triggers + all PSUM evacuation; Sync = all x/xp load triggers.
"""

import sys

import numpy as np

_TRN_REPO = "/opt/trn_rl_repo"
if _TRN_REPO not in sys.path:
    sys.path.insert(0, _TRN_REPO)

B, H, S, D = 4, 32, 4096, 64
BH = B * H                      # 128 (b,h) slabs
N_CORES = 8
BH_PER_CORE = BH // N_CORES     # 16 slabs per core
KD = 14                         # slabs on the DVE path
KP = BH_PER_CORE - KD           # slabs on the PE path
FREE = (S // 128) * D           # 2048 free elements per partition per slab
ROWS_D = KD * 128               # dve-path dram rows per core
PF = S // 2                     # 2048 pe-path free elems per slab (=64q+32m grid)
CHUNK_PLAN = [1, 1, 2, 4, 4, 1, 1]
assert sum(CHUNK_PLAN) == KD
U_BUFS = 2
OUT_SCALE = 8.0                 # int8 code = out * 127/OUT_SCALE
THETA = 10000.0

_prog_cache = {}


def _build_program():
    """Build (and cache) the SPMD Bass program for one core."""
    if "nc" in _prog_cache:
        return _prog_cache["nc"]

    import concourse.bacc as bacc
    import concourse.bass as bass
    import concourse.mybir as mybir
    import concourse.tile as tile

    f16 = mybir.dt.float16
    f32 = mybir.dt.float32
    i8 = mybir.dt.int8
    nc = bacc.Bacc(
        "TRN2", target_bir_lowering=False, debug=False, num_devices=N_CORES
    )
    x_d = nc.dram_tensor("x", [ROWS_D, FREE], f16, kind="ExternalInput").ap()
    xp_d = nc.dram_tensor("xp", [128, KP * FREE], f16, kind="ExternalInput").ap()
    ta_d = nc.dram_tensor("ta", [128, FREE], f16, kind="ExternalInput").ap()
    tb_d = nc.dram_tensor("tb", [128, FREE], f16, kind="ExternalInput").ap()
    wa_d = nc.dram_tensor("wa", [128, 32 * 128], f16, kind="ExternalInput").ap()
    wb_d = nc.dram_tensor("wb", [128, 64 * 128], f16, kind="ExternalInput").ap()
    o_d = nc.dram_tensor("out", [ROWS_D, FREE], f16, kind="ExternalOutput").ap()
    op_d = nc.dram_tensor("op", [128, KP * FREE], i8, kind="ExternalOutput").ap()

    def store_engine():
        # only the SP (sync) and Activation (scalar) rings are HWDGE-capable.
        # sync must carry ONLY x loads (a store trigger waits on its data and
        # would block later load triggers in the in-order engine queue), so
        # stores, tables, weights and xp all go on the scalar ring in rough
        # chronological order of readiness.
        return nc.scalar

    with tile.TileContext(nc) as tc:
        with (
            tc.tile_pool(name="tabs", bufs=1) as tabs,
            tc.tile_pool(name="xbig", bufs=1) as xbigp,
            tc.tile_pool(name="u", bufs=U_BUFS) as upool,
            tc.tile_pool(name="obig", bufs=1) as obigp,
            tc.tile_pool(name="xpb", bufs=1) as xpbp,
            tc.tile_pool(name="wt", bufs=1) as wtp,
            tc.tile_pool(name="y2", bufs=1) as y2p,
            tc.tile_pool(name="o8p", bufs=1) as o8pp,
            tc.tile_pool(name="ps", bufs=2, space="PSUM") as psp,
        ):
            # ---- table/weight loads: scalar HWDGE ring (idle at start) ----
            tb = tabs.tile([128, FREE], f16)
            ta = tabs.tile([128, FREE], f16)
            hf = FREE // 2
            nc.scalar.dma_start(tb[:, :hf], tb_d[:, :hf])
            nc.scalar.dma_start(ta[:, :hf], ta_d[:, :hf])
            wa = wtp.tile([128, 32 * 128], f16)
            wb = wtp.tile([128, 64 * 128], f16)

            xbig = xbigp.tile([128, KD * FREE], f16)
            obig = obigp.tile([128, KD * FREE], f16)
            xpb = xpbp.tile([128, KP * FREE], f16)
            y2 = y2p.tile([128, KP * FREE], f16)
            o8p = o8pp.tile([128, KP * FREE], i8)

            def compute(xoff, ut, nsl, lo, sz):
                """u = x*tb; o = x*ta; o += pairswap(u) (fp16 out)."""
                if nsl == 1:
                    xs = xbig[:, xoff + lo : xoff + lo + sz]
                    us = ut[:, lo : lo + sz]
                    os_ = obig[:, xoff + lo : xoff + lo + sz]
                    nc.vector.tensor_mul(us, xs, tb[:, lo : lo + sz])
                    nc.vector.tensor_mul(os_, xs, ta[:, lo : lo + sz])
                else:
                    assert lo == 0 and sz == nsl * FREE
                    cf = nsl * FREE
                    x3 = xbig[:, xoff : xoff + cf].rearrange(
                        "p (j f) -> p j f", j=nsl
                    )
                    u3 = ut[:].rearrange("p (j f) -> p j f", j=nsl)
                    os_ = obig[:, xoff : xoff + cf]
                    o3 = os_.rearrange("p (j f) -> p j f", j=nsl)
                    ta_b = bass.AP(
                        ta[:].tensor, ta[:].offset,
                        [ta[:].ap[0], [0, nsl], ta[:].ap[1]],
                    )
                    tb_b = bass.AP(
                        tb[:].tensor, tb[:].offset,
                        [tb[:].ap[0], [0, nsl], tb[:].ap[1]],
                    )
                    nc.vector.tensor_mul(u3, x3, tb_b)
                    nc.vector.tensor_mul(o3, x3, ta_b)
                    us = ut[:, :sz]
                usw = us.rearrange("p (n two) -> p n two", two=2)[:, :, ::-1]
                os3 = os_.rearrange("p (n two) -> p n two", two=2)
                nc.vector.tensor_add(os3, os3, usw)

            # ---- emission: interleave DVE chunks with PE pass A ----
            def emit_dve_chunk(ci, row0, nsl):
                first = ci == 0
                cfree = nsl * FREE
                xoff = row0 * FREE
                rows = x_d[row0 * 128 : (row0 + nsl) * 128, :]
                xts = xbig[:, xoff : xoff + cfree]
                if first:
                    assert nsl == 1
                    h = cfree // 2
                    nc.sync.dma_start(xts[:, :h], rows[:, :h])
                    nc.sync.dma_start(xts[:, h:], rows[:, h:])
                else:
                    src = rows.rearrange("(j p) f -> p j f", j=nsl)
                    nc.sync.dma_start(
                        xts.rearrange("p (j f) -> p j f", j=nsl), src
                    )
                ut = upool.tile([128, cfree], f16, tag="u")
                o8ts = obig[:, xoff : xoff + cfree]
                orows = o_d[row0 * 128 : (row0 + nsl) * 128, :]
                if first:
                    h = cfree // 2
                    for hi in range(2):
                        compute(xoff, ut, 1, hi * h, h)
                        if hi == 0:
                            # second table halves after first half-compute
                            nc.sync.dma_start(tb[:, hf:], tb_d[:, hf:])
                            nc.sync.dma_start(ta[:, hf:], ta_d[:, hf:])
                        store_engine().dma_start(
                            orows[:, hi * h : (hi + 1) * h],
                            o8ts[:, hi * h : (hi + 1) * h],
                        )
                else:
                    compute(xoff, ut, nsl, 0, cfree)
                    dst = orows.rearrange("(j p) f -> p j f", j=nsl)
                    store_engine().dma_start(
                        dst, o8ts.rearrange("p (j f) -> p j f", j=nsl)
                    )

            def emit_xp_load(k):
                nc.scalar.dma_start(
                    xpb[:, k * FREE : (k + 1) * FREE],
                    xp_d[:, k * FREE : (k + 1) * FREE],
                )

            def emit_passA(k):
                """32 matmuls (N=64) for slab k into two half PSUM tiles,
                each followed by a ScalarE fp32->fp16 evacuation into y2."""
                for hh in range(2):
                    pst = psp.tile([128, 1024], f32, tag="psA")
                    for mi in range(16):
                        m = 16 * hh + mi
                        rhs = xpb[
                            :, k * FREE + m * 64 : k * FREE + (m + 1) * 64
                        ]
                        nc.tensor.matmul(
                            pst[:, mi * 64 : (mi + 1) * 64],
                            lhsT=wa[:, m * 128 : (m + 1) * 128],
                            rhs=rhs,
                            start=True,
                            stop=True,
                        )
                    # flat evacuation: y2 keeps the (m, q) PSUM layout;
                    # pass B absorbs the transpose in its rhs AP (strided
                    # column addressing is free for the PE moving operand)
                    dst = y2[:, k * FREE + hh * 1024 : k * FREE + (hh + 1) * 1024]
                    nc.scalar.copy(dst, pst[:])

            # merged schedule. The scalar ring carries tables, weights,
            # xp and all stores; a store trigger blocks the ScE queue until
            # its data is ready, so trigger emission position doubles as a
            # timer that keeps weight/xp transfers from stealing load
            # bandwidth from the x chunks the DVE needs next.
            dve_sched = []
            row0 = 0
            for ci, nsl in enumerate(CHUNK_PLAN):
                dve_sched.append((ci, row0, nsl))
                row0 += nsl
            emit_dve_chunk(*dve_sched[0])
            emit_dve_chunk(*dve_sched[1])
            emit_dve_chunk(*dve_sched[2])
            nc.scalar.dma_start(wa[:], wa_d[:])
            emit_xp_load(0)
            emit_xp_load(1)
            emit_passA(0)
            nc.scalar.dma_start(wb[:], wb_d[:])
            emit_dve_chunk(*dve_sched[3])
            emit_passA(1)

            y2v = y2[:].rearrange("p (k m q) -> p k m q", k=KP, m=32)
            for g in range(8):
                pst = psp.tile([128, 512], f32, tag="psB")
                for q2 in range(8):
                    q = 8 * g + q2
                    rhs = y2v[:, :, :, q]  # columns (k, m), strided
                    nc.tensor.matmul(
                        pst[:, q2 * 64 : (q2 + 1) * 64],
                        lhsT=wb[:, q * 128 : (q + 1) * 128],
                        rhs=rhs,
                        start=True,
                        stop=True,
                    )
                # evac fp32 -> int8 codes, flat
                nc.scalar.copy(o8p[:, g * 512 : (g + 1) * 512], pst[:])
                if g % 4 == 3:
                    w = g // 4
                    store_engine().dma_start(
                        op_d[:, w * 2048 : (w + 1) * 2048],
                        o8p[:, w * 2048 : (w + 1) * 2048],
                    )

            emit_dve_chunk(*dve_sched[4])
            emit_dve_chunk(*dve_sched[5])
            emit_dve_chunk(*dve_sched[6])

    nc.compile()
    _prog_cache["nc"] = nc
    return nc


def _default_rotate_m(theta=THETA):
    """Rebuild the reference's rotation buffer if the harness doesn't pass it."""
    half = D // 2
    try:  # replicate the reference's jax-f32 arithmetic exactly if possible
        import jax.numpy as jnp

        pos = np.asarray(jnp.arange(S, dtype=jnp.float32))
        inv_freq = np.asarray(
            theta ** (-(2.0 * jnp.arange(half, dtype=jnp.float32)) / D)
        )
        ang = np.asarray(pos[:, None] * inv_freq[None, :], dtype=np.float32)
        c, s = np.asarray(jnp.cos(ang)), np.asarray(jnp.sin(ang))
    except Exception:
        pos = np.arange(S, dtype=np.float32)
        exp = (-(2.0 * np.arange(half, dtype=np.float32)) / D).astype(np.float32)
        inv_freq = np.power(np.float32(theta), exp, dtype=np.float32)
        ang = (pos[:, None] * inv_freq[None, :]).astype(np.float32)
        c, s = np.cos(ang, dtype=np.float32), np.sin(ang, dtype=np.float32)
    idx = 2 * np.arange(half)
    r = np.zeros((S, D, D), dtype=np.float32)
    r[:, idx, idx] = c
    r[:, idx, idx + 1] = -s
    r[:, idx + 1, idx] = s
    r[:, idx + 1, idx + 1] = c
    return r


def _tables(token_positions, rotate_m):
    """Host-precompute the [128, FREE] fp16 A and B' tables for the DVE
    path (scaled by 127/OUT_SCALE) — see baseline docstring for layout."""
    if rotate_m is None:
        rotate_m = _default_rotate_m()
    r = np.asarray(rotate_m, dtype=np.float32)[np.asarray(token_positions)]
    idx = np.arange(D // 2) * 2
    a = r[:, idx, idx]
    b = r[:, idx, idx + 1]
    c = r[:, idx + 1, idx + 1]
    d = r[:, idx + 1, idx]
    A = np.empty((S, D), np.float32)
    A[:, 0::2] = a
    A[:, 1::2] = c
    Bp = np.empty((S, D), np.float32)
    Bp[:, 0::2] = d
    Bp[:, 1::2] = b
    return (
        np.ascontiguousarray(A.reshape(128, FREE)).astype(np.float16),
        np.ascontiguousarray(Bp.reshape(128, FREE)).astype(np.float16),
    )


def _rot_T(angles):
    """[n] angles -> [n, 64, 64] fp32 block-diag R^T tiles (R^T[i, o])."""
    n = angles.shape[0]
    half = D // 2
    inv = (THETA ** (-(2.0 * np.arange(half, dtype=np.float32)) / D)).astype(
        np.float32
    )
    ang = angles[:, None].astype(np.float32) * inv[None, :]
    c, s = np.cos(ang), np.sin(ang)
    rT = np.zeros((n, D, D), np.float32)
    idx = 2 * np.arange(half)
    # R[o,i]: [2k,2k]=c [2k,2k+1]=-s [2k+1,2k]=s [2k+1,2k+1]=c; rT[i,o]=R[o,i]
    rT[:, idx, idx] = c
    rT[:, idx + 1, idx] = -s
    rT[:, idx, idx + 1] = s
    rT[:, idx + 1, idx + 1] = c
    return rT


def _pe_weights():
    """wa: [128, 32*128] fp16, tile m = blockdiag(R_{2m}^T, R_{2m+1}^T).
    wb: [128, 64*128] fp16 (scaled), tile q = blockdiag(R_{64q}^T, same)."""
    ra = _rot_T(np.arange(64, dtype=np.float32))          # [64,64,64] r = 0..63
    rb = _rot_T(64.0 * np.arange(64, dtype=np.float32))   # 64q
    wa = np.zeros((128, 32, 128), np.float32)
    for m in range(32):
        wa[:64, m, :64] = ra[2 * m]
        wa[64:, m, 64:] = ra[2 * m + 1]
    wb = np.zeros((128, 64, 128), np.float32)
    for q in range(64):
        wb[:64, q, :64] = rb[q]
        wb[64:, q, 64:] = rb[q]
    kq = np.float32(127.0 / OUT_SCALE)
    return (
        np.ascontiguousarray(wa.reshape(128, 32 * 128)).astype(np.float16),
        np.ascontiguousarray(wb.reshape(128, 64 * 128) * kq).astype(np.float16),
    )


def _pe_maps(token_positions):
    """Index maps between (s, i) and the PE layouts (general permutation tp)."""
    tp = np.asarray(token_positions).astype(np.int64)
    inv = np.empty(S, np.int64)
    inv[tp] = np.arange(S)
    p = np.arange(128)
    jrow, irow = p // 64, p % 64
    f = np.arange(PF)
    mcol, qcol = f // 64, f % 64
    # xp[p, k*PF + f] = x[k][ s2[p, f], irow[p] ]
    s2 = inv[64 * qcol[None, :] + 2 * mcol[None, :] + jrow[:, None]]  # [128, PF]
    # out decode: per position s: (q, m, j) of tp[s]
    qq, rr = np.divmod(tp, 64)
    mm, jj = np.divmod(rr, 2)
    return s2, irow, (qq, mm, jj)


def _in_maps(x, token_positions, rotate_m):
    ta, tb = _tables(token_positions, rotate_m)
    wa, wb = _pe_weights()
    s2, irow, _ = _pe_maps(token_positions)
    xs = np.asarray(x, dtype=np.float32).astype(np.float16).reshape(
        N_CORES, BH_PER_CORE, S, D
    )
    maps = []
    for cc in range(N_CORES):
        xd = np.ascontiguousarray(
            xs[cc, :KD].reshape(ROWS_D, FREE)
        )
        xpm = np.empty((128, KP, PF), np.float16)
        for k in range(KP):
            xpm[:, k, :] = xs[cc, KD + k][s2, irow[:, None]]
        maps.append(
            {
                "x": xd,
                "xp": np.ascontiguousarray(xpm.reshape(128, KP * PF)),
                "ta": ta,
                "tb": tb,
                "wa": wa,
                "wb": wb,
            }
        )
    return maps


def _run(x, token_positions, rotate_m=None, trace=False, trace_cores=None):
    from concourse.bass_utils import run_bass_kernel_spmd

    nc = _build_program()
    in_maps = _in_maps(x, token_positions, rotate_m)
    res = run_bass_kernel_spmd(
        nc,
        in_maps,
        list(range(N_CORES)),
        trace=trace,
        trace_cores=trace_cores,
    )
    _, _, (qq, mm, jj) = _pe_maps(token_positions)
    dq = np.float32(OUT_SCALE / 127.0)
    oar = np.arange(D)
    rows_idx = jj[:, None] * 64 + oar[None, :]          # [S, D]
    out = np.empty((N_CORES, BH_PER_CORE, S, D), np.float32)
    for cc in range(N_CORES):
        od = res.results[cc]["out"].reshape(KD, 128, FREE)
        out[cc, :KD] = od.reshape(KD, S, D)
        op = res.results[cc]["op"]                       # [128, KP*PF] int8
        for k in range(KP):
            cols_idx = qq * (32 * KP) + k * 32 + mm      # [S]
            out[cc, KD + k] = op[rows_idx, cols_idx[:, None]].astype(
                np.float32
            ) * dq
    return out.reshape(B, H, S, D), res


def kernel(x, token_positions, rotate_m=None, **_unused):
    out, _ = _run(x, token_positions, rotate_m, trace=False)
    return out


if __name__ == "__main__":
    _build_program()
    print("compiled OK")


# revision 19
# speedup vs baseline: 1.1895x; 1.1895x over previous
"""MetaRoPE kernel for Trainium2, 8 NeuronCores — fp16 I/O + DVE 2x-mode.

Reference computation:
    r = rotate_m[token_positions]            # [S, D, D], block-diag 2x2 rotations
    out = einsum('bhsi,soi->bhso', x, r)     # x: [4, 32, 4096, 64] fp32

Because r is block-diagonal with 2x2 blocks, for each position s and pair k:
    out[2k]   = a*x[2k] + b*x[2k+1]     (a = r[2k,2k],   b = r[2k,2k+1])
    out[2k+1] = c*x[2k+1] + d*x[2k]     (c = r[2k+1,2k+1], d = r[2k+1,2k])
which we compute elementwise as
    out = x * A + pairswap(x * B')
with host-precomputed tables A, B' of shape [S, D]:
    A[s,2k] = a, A[s,2k+1] = c
    B'[s,2k] = d, B'[s,2k+1] = b       (B' is pre-pairswapped so that
                                        pairswap(x*B') lands b*x_odd on even
                                        lanes and d*x_even on odd lanes)

Precision/bandwidth: the correctness gate is rel_err < 2e-2; fp16 end-to-end
(host converts x fp32->fp16, device computes in fp16, host converts the fp16
result back) measures ~1.1e-3 and halves both HBM traffic and DVE element
cost vs fp32. Plain InstTensorTensor ops hit the DVE 2x_1p perf mode with
packed fp16 (~0.5 ns/elem/partition measured, including the stride -1
pair-swap operand). Notes from measurement on HW:
  - scalar_tensor_tensor (fused 3-input op) supports NO DVE perf modes and
    runs ~1.2 ns/elem — slower than two plain ops.
  - 4-dim merged APs (one mul writing u and o via a broadcast x) run ~15%
    slower per element than 3-dim APs.
  - GpSimd tensor ops are Q7 software (~2.5-6.4 ns/elem) AND degrade
    concurrent DVE throughput via SBUF contention — never offload to it.
  - TENSOR_TENSOR with an int8 output drops to 1x mode (+17 us measured):
    int8 output needs a separate cast, which costs more than it saves here.
  - Stores measure only ~142 GB/s/core (loads ~314 GB/s/core) regardless
    of ring count, burst size or layout; loads and stores overlap fully
    (duplex), but loads split across BOTH rings still cap ~320 GB/s total.
  - The idle TensorE cannot profitably offload elementwise work: matmul
    output is PSUM-fp32-only on TRN2, and evacuating PSUM costs ~1.3
    ns/elem (ScalarE flat ACTIVATE copy; 4.9 ns/elem if the copy AP is
    strided) per pass — more than the ~1.6 ns/elem the DVE 3-pass spends
    outright. A full two-round block-diag-matmul hybrid (R_s = R_{64q} R_r,
    96 weight tiles, 14 DVE + 2 PE slabs) was built and measured 75-119 us
    across schedules: the extra ~4 MB of weight/relayout loads starves the
    DVE mid-stream at the shared load ceiling (see kernel_hybrid_pe.py.bak).

Sharding: x reshaped to [128 (b,h) slabs, 4096, 64]; 16 slabs per core.
Each slab [4096*64] is viewed as [128 partitions, 2048 free] (contiguous per
partition; partition p holds positions 32p..32p+31). Tables are replicated
to every core as [128, 2048] fp16 tiles that match that layout for every
slab.

Per core the 16 slabs are processed in chunks (CHUNK_PLAN, tapered small at
the ends to shrink pipeline ramp/tail; 4-slab middle chunks cut the DVE
instruction count ~42 -> ~30, saving ~1.6 us vs the all-2-slab plan). Each
chunk: one load (HWDGE on the sync ring), two DVE tensor_muls (tables
broadcast across the chunk's slabs via a step-0 AP dim) + one pair-swapped
in-place tensor_add, one store (HWDGE on the scalar ring). Table halves
split across rings: first halves on the scalar ring up front, second halves
on the sync ring emitted after the head chunk's first half-compute, so that
compute (which needs only table cols [0:1024)) starts ~3.5 us earlier —
tile deps follow emission order, and every DVE op emitted after a DMA to
tb/ta waits on it.

Measured: 72.5 us HW exec (from the 149.8 us fp32 baseline; 74.3 us with
all-2-slab middle chunks), rel err 1.15e-3. Breakdown: ~6.9 us fixed
engine init before the first DMA trigger, ~11.5 us to first TT (init +
table/first-chunk DMA latency), ~50-58 us DVE busy (51.2 us fp16-2x
compute roofline + semaphore slices + per-instr overhead), ~3 us tail
(final store + barrier). The DVE stream and the 142 GB/s store path
finish within ~2 us of each other — co-limited; int8 stores or TensorE
offload each failed to beat this point (notes above).
"""

import sys

import numpy as np

_TRN_REPO = "/opt/trn_rl_repo"
if _TRN_REPO not in sys.path:
    sys.path.insert(0, _TRN_REPO)

B, H, S, D = 4, 32, 4096, 64
BH = B * H                      # 128 (b,h) slabs
N_CORES = 8
BH_PER_CORE = BH // N_CORES     # 16 slabs per core
FREE = (S // 128) * D           # 2048 free elements per partition per slab
ROWS = BH_PER_CORE * 128        # 2048 dram rows per core, [ROWS, FREE] fp16
# slabs per chunk, tapered: small first chunks so compute starts early,
# small last chunk so the final store is short
CHUNK_PLAN = [1, 1, 2, 4, 4, 2, 1, 1]
assert sum(CHUNK_PLAN) == BH_PER_CORE
U_BUFS = 2

_prog_cache = {}


def _build_program():
    """Build (and cache) the SPMD Bass program for one core."""
    if "nc" in _prog_cache:
        return _prog_cache["nc"]

    import concourse.bacc as bacc
    import concourse.bass as bass
    import concourse.mybir as mybir
    import concourse.tile as tile

    f16 = mybir.dt.float16
    nc = bacc.Bacc(
        "TRN2", target_bir_lowering=False, debug=False, num_devices=N_CORES
    )
    x_d = nc.dram_tensor("x", [ROWS, FREE], f16, kind="ExternalInput").ap()
    ta_d = nc.dram_tensor("ta", [128, FREE], f16, kind="ExternalInput").ap()
    tb_d = nc.dram_tensor("tb", [128, FREE], f16, kind="ExternalInput").ap()
    o_d = nc.dram_tensor("out", [ROWS, FREE], f16, kind="ExternalOutput").ap()

    with tile.TileContext(nc) as tc:
        with (
            tc.tile_pool(name="tabs", bufs=1) as tabs,
            tc.tile_pool(name="xbig", bufs=1) as xbigp,
            tc.tile_pool(name="u", bufs=U_BUFS) as upool,
            tc.tile_pool(name="obig", bufs=1) as obigp,
        ):
            # table loads go on the scalar HWDGE ring (idle at start) so
            # they overlap the first x-chunk load on the sync ring; halves
            # ordered so the first half-slab compute (needs tb+ta cols
            # [0:hf)) can start before the full tables land
            tb = tabs.tile([128, FREE], f16)
            ta = tabs.tile([128, FREE], f16)
            hf = FREE // 2
            # NOTE: only the SP (sync) and Activation (scalar) rings are
            # HWDGE-capable here — a third queue via nc.vector.dma_start is
            # rejected by bass, and gpsimd SWDGE pays software desc-gen.
            # So the two first halves serialize on the scalar ring; quarter
            # pieces and ring-splitting were both tried and did not move
            # the first multiply earlier.
            nc.scalar.dma_start(tb[:, :hf], tb_d[:, :hf])
            nc.scalar.dma_start(ta[:, :hf], ta_d[:, :hf])
            # second halves are loaded from inside the chunk loop (on the
            # sync ring, after the first x chunk) so the head chunk's first
            # half-compute — emitted before them — only depends on the
            # first-half table loads (tile deps follow emission order).
            # (Tried instead putting ta's first half on the sync ring ahead
            # of x: per-queue transfer serialization pushed the first x half
            # later and the first multiply slipped ~2 us — keep both first
            # halves on the scalar ring.)

            # x and out live in single whole-core SBUF buffers (64 KiB per
            # partition each): every slice is written once and read once, so
            # there are no tile-reuse WAR waits — the only semaphores left
            # are load-done -> mul and add-done -> store per chunk
            xbig = xbigp.tile([128, BH_PER_CORE * FREE], f16)
            obig = obigp.tile([128, BH_PER_CORE * FREE], f16)

            def compute(xoff, ut, nsl, lo, sz):
                """u = x*tb; o = x*ta; o += pairswap(u) on cols [lo, lo+sz)
                of each of the nsl slabs at element offset xoff in the big
                x/out buffers (3-dim APs throughout)."""
                if nsl == 1:
                    xs = xbig[:, xoff + lo : xoff + lo + sz]
                    us = ut[:, lo : lo + sz]
                    os_ = obig[:, xoff + lo : xoff + lo + sz]
                    nc.vector.tensor_mul(us, xs, tb[:, lo : lo + sz])
                    nc.vector.tensor_mul(os_, xs, ta[:, lo : lo + sz])
                else:
                    assert lo == 0 and sz == nsl * FREE
                    cf = nsl * FREE
                    x3 = xbig[:, xoff : xoff + cf].rearrange(
                        "p (j f) -> p j f", j=nsl
                    )
                    u3 = ut[:].rearrange("p (j f) -> p j f", j=nsl)
                    os_ = obig[:, xoff : xoff + cf]
                    o3 = os_.rearrange("p (j f) -> p j f", j=nsl)
                    ta_b = bass.AP(
                        ta[:].tensor, ta[:].offset,
                        [ta[:].ap[0], [0, nsl], ta[:].ap[1]],
                    )
                    tb_b = bass.AP(
                        tb[:].tensor, tb[:].offset,
                        [tb[:].ap[0], [0, nsl], tb[:].ap[1]],
                    )
                    nc.vector.tensor_mul(u3, x3, tb_b)
                    nc.vector.tensor_mul(o3, x3, ta_b)
                    us = ut[:]
                usw = us.rearrange("p (n two) -> p n two", two=2)[:, :, ::-1]
                os3 = os_.rearrange("p (n two) -> p n two", two=2)
                nc.vector.tensor_add(os3, os3, usw)

            row0 = 0
            for ci, nsl in enumerate(CHUNK_PLAN):
                first = ci == 0
                last = ci == len(CHUNK_PLAN) - 1
                cfree = nsl * FREE
                xoff = row0 * FREE
                rows = x_d[row0 * 128 : (row0 + nsl) * 128, :]
                xts = xbig[:, xoff : xoff + cfree]
                if first:
                    # split the first load so compute can start after 0.25 MiB
                    assert nsl == 1
                    h = cfree // 2
                    nc.sync.dma_start(xts[:, :h], rows[:, :h])
                    nc.sync.dma_start(xts[:, h:], rows[:, h:])
                else:
                    src = rows.rearrange("(j p) f -> p j f", j=nsl)
                    nc.sync.dma_start(
                        xts.rearrange("p (j f) -> p j f", j=nsl), src
                    )

                ut = upool.tile([128, cfree], f16, tag="u")
                ots = obig[:, xoff : xoff + cfree]
                orows = o_d[row0 * 128 : (row0 + nsl) * 128, :]

                if first or last:
                    # head chunk in halves: starts computing after the first
                    # half-load. Tail chunk in quarters: the final store is
                    # only 0.125 MB, shrinking the end-of-run barrier wait
                    # for it (~1 us off the measured span).
                    nparts = 2 if first else 4
                    h = cfree // nparts
                    for hi in range(nparts):
                        compute(xoff, ut, 1, hi * h, h)
                        if first and hi == 0:
                            # second table halves, after the head's first
                            # half-compute in emission order
                            nc.sync.dma_start(tb[:, hf:], tb_d[:, hf:])
                            nc.sync.dma_start(ta[:, hf:], ta_d[:, hf:])
                        nc.scalar.dma_start(
                            orows[:, hi * h : (hi + 1) * h],
                            ots[:, hi * h : (hi + 1) * h],
                        )
                else:
                    compute(xoff, ut, nsl, 0, cfree)
                    dst = orows.rearrange("(j p) f -> p j f", j=nsl)
                    nc.scalar.dma_start(
                        dst, ots.rearrange("p (j f) -> p j f", j=nsl)
                    )
                row0 += nsl

    nc.compile()
    _prog_cache["nc"] = nc
    return nc


def _default_rotate_m(theta=10000.0):
    """Rebuild the reference's rotation buffer if the harness doesn't pass it."""
    half = D // 2
    try:  # replicate the reference's jax-f32 arithmetic exactly if possible
        import jax.numpy as jnp

        pos = np.asarray(jnp.arange(S, dtype=jnp.float32))
        inv_freq = np.asarray(
            theta ** (-(2.0 * jnp.arange(half, dtype=jnp.float32)) / D)
        )
        ang = np.asarray(pos[:, None] * inv_freq[None, :], dtype=np.float32)
        c, s = np.asarray(jnp.cos(ang)), np.asarray(jnp.sin(ang))
    except Exception:
        pos = np.arange(S, dtype=np.float32)
        exp = (-(2.0 * np.arange(half, dtype=np.float32)) / D).astype(np.float32)
        inv_freq = np.power(np.float32(theta), exp, dtype=np.float32)
        ang = (pos[:, None] * inv_freq[None, :]).astype(np.float32)
        c, s = np.cos(ang, dtype=np.float32), np.sin(ang, dtype=np.float32)
    idx = 2 * np.arange(half)
    r = np.zeros((S, D, D), dtype=np.float32)
    r[:, idx, idx] = c
    r[:, idx, idx + 1] = -s
    r[:, idx + 1, idx] = s
    r[:, idx + 1, idx + 1] = c
    return r


def _tables(token_positions, rotate_m):
    """Host-precompute the [128, FREE] fp16 A and B' tables (see docstring)."""
    if rotate_m is None:
        rotate_m = _default_rotate_m()
    r = np.asarray(rotate_m, dtype=np.float32)[np.asarray(token_positions)]
    idx = np.arange(D // 2) * 2
    a = r[:, idx, idx]            # x_even -> out_even
    b = r[:, idx, idx + 1]        # x_odd  -> out_even
    c = r[:, idx + 1, idx + 1]    # x_odd  -> out_odd
    d = r[:, idx + 1, idx]        # x_even -> out_odd
    A = np.empty((S, D), np.float32)
    A[:, 0::2] = a
    A[:, 1::2] = c
    Bp = np.empty((S, D), np.float32)
    Bp[:, 0::2] = d
    Bp[:, 1::2] = b
    return (
        np.ascontiguousarray(A.reshape(128, FREE)).astype(np.float16),
        np.ascontiguousarray(Bp.reshape(128, FREE)).astype(np.float16),
    )


def _in_maps(x, token_positions, rotate_m):
    ta, tb = _tables(token_positions, rotate_m)
    xs = np.asarray(x, dtype=np.float32).astype(np.float16).reshape(
        N_CORES, ROWS, FREE
    )
    xs = np.ascontiguousarray(xs)
    return [{"x": xs[i], "ta": ta, "tb": tb} for i in range(N_CORES)]


def _run(x, token_positions, rotate_m=None, trace=False, trace_cores=None):
    from concourse.bass_utils import run_bass_kernel_spmd

    nc = _build_program()
    in_maps = _in_maps(x, token_positions, rotate_m)
    res = run_bass_kernel_spmd(
        nc,
        in_maps,
        list(range(N_CORES)),
        trace=trace,
        trace_cores=trace_cores,
    )
    out = np.concatenate(
        [res.results[i]["out"].reshape(1, ROWS * FREE) for i in range(N_CORES)]
    ).reshape(B, H, S, D).astype(np.float32)
    return out, res


def kernel(x, token_positions, rotate_m=None, **_unused):
    out, _ = _run(x, token_positions, rotate_m, trace=False)
    return out
